# revision 8
# baseline (speedup 1.0000x reference)
"""APPNP (MLP + K-step personalized-pagerank propagation) on 8 TRN2 NeuronCores.

Strategy:
  * Nodes are relabeled into a per-core "class layout": each core owns 12500
    destination nodes; per node the (self-loop-inclusive) degree is padded to
    a multiple of 2 and nodes are grouped into classes by padded degree.
  * norm = dinv[src]*dinv[dst] factorizes, so the propagated table holds
    h_scaled = dinv*h (bf16) and the per-step blend is two elementwise ops.
  * Per step: AllGather the bf16 table shards (DRAM collective), gather each
    edge-slot row with one-index-per-partition indirect DMAs (128 slots per
    call), pair-add + fold on the Vector engine (segment sums), blend, repeat.
  * The 3-layer MLP runs feature-major in bf16 on the TensorEngine with fp32
    PSUM accumulation; outputs are transposed back to node-major via PE.

All graph preprocessing (sorting, padding, index tables) is host-side numpy;
indices are step-invariant and stay resident in SBUF.
"""

import numpy as np
import ml_dtypes

N = 100000
E = 1600000
NFEAT = 500
NCLASS = 40
K = 10
ALPHA = 0.1
NCORES = 8
SH = N // NCORES          # real dsts per core
D = NCLASS
P = 128

bf16 = ml_dtypes.bfloat16

# --------------------------------------------------------------------------
# compat patch: this walrus build rejects >1 sync-wait per instruction.
# Hoist excess waits onto standalone EventSemaphore instructions.
# --------------------------------------------------------------------------
_PATCHED = False


def _install_wait_split():
    global _PATCHED
    if _PATCHED:
        return
    import orjson
    import concourse.bass as _bass

    _orig = _bass.Bass.to_json_bytes

    def _patched(self):
        j = orjson.loads(_orig(self))
        for func in j.get("functions", []):
            for blk in func.get("blocks", []):
                insts = blk.get("instructions")
                if not insts:
                    continue
                out = []
                for inst in insts:
                    si = inst.get("sync_info") or {}
                    waits = si.get("on_wait") or []
                    if len(waits) > 1:
                        for k2, w in enumerate(waits[:-1]):
                            out.append({
                                "debug": inst.get("debug", 0),
                                "engine": inst["engine"],
                                "ins": [],
                                "name": f"{inst['name']}_sw{k2}",
                                "opcode": "EventSemaphore",
                                "outs": [],
                                "sync_info": {"on_update": [], "on_wait": [w]},
                            })
                        si["on_wait"] = [waits[-1]]
                        inst["sync_info"] = si
                    out.append(inst)
                blk["instructions"] = out
        return orjson.dumps(j)

    _bass.Bass.to_json_bytes = _patched
    _PATCHED = True


# --------------------------------------------------------------------------
# host-side graph preprocessing
# --------------------------------------------------------------------------
def _preprocess(edge_index, x, W0, b0, W1, b1, W2, b2):
    src = np.concatenate([edge_index[0], np.arange(N, dtype=np.int64)])
    dst = np.concatenate([edge_index[1], np.arange(N, dtype=np.int64)])
    deg = np.bincount(dst, minlength=N).astype(np.int64)       # >= 1
    dinv = (1.0 / np.sqrt(deg.astype(np.float32))).astype(np.float32)

    order = np.argsort(dst, kind="stable")
    src_s = src[order].astype(np.int64)
    segstart = np.zeros(N + 1, np.int64)
    np.cumsum(deg, out=segstart[1:])

    deg_g = deg - 1                        # gather degree (self-loop folded on-chip)
    m_of = (deg_g + 1) // 2                # class id per node (0 = no gather)
    maxm = int(m_of.max())
    # merge sparse classes upward (block-roundup waste dominates small classes)
    cg = np.bincount(m_of, minlength=maxm + 1)
    kept = [m for m in range(1, maxm + 1) if cg[m] >= 1024]
    if not kept or kept[-1] != maxm:
        kept.append(maxm)
    remap = np.zeros(maxm + 1, np.int64)
    for m in range(1, maxm + 1):
        remap[m] = next((kk for kk in kept if kk >= m), maxm)
    m_of = np.where(m_of >= 1, remap[np.minimum(m_of, maxm)], 0)

    # class-balanced dst->core assignment: deal each class round-robin
    cores_of = np.zeros(N, np.int64)
    k_m = np.zeros(maxm + 1, np.int64)
    global_class = [None] * (maxm + 1)
    for m in range(0, maxm + 1):
        lst = np.where(m_of == m)[0]
        global_class[m] = lst
        cores_of[lst] = np.arange(len(lst)) % NCORES
        per_core_max = (len(lst) + NCORES - 1) // NCORES
        k_m[m] = int(np.ceil(per_core_max / P))
    k_m[0] += 1                                # forced all-dummy block (zero rows)
    while int(k_m.sum()) % 4 != 0:
        k_m[0] += 1
    K_TOT = int(k_m.sum())
    SHPAD = P * K_TOT
    j_off = np.zeros(maxm + 1, np.int64)
    acc = 0
    for m in range(0, maxm + 1):
        j_off[m] = acc
        acc += k_m[m]

    # assignment: J[v], Pp[v]
    J = np.zeros(N, np.int64)
    Pp = np.zeros(N, np.int64)
    class_lists = [[None] * (maxm + 1) for _ in range(NCORES)]
    for m in range(0, maxm + 1):
        lst = global_class[m]
        for c in range(NCORES):
            sub = lst[cores_of[lst] == np.int64(c)]
            class_lists[c][m] = sub
            t = np.arange(len(sub))
            J[sub] = j_off[m] + t // P
            Pp[sub] = t % P
    rowid = cores_of * SHPAD + J * P + Pp      # global table row per node

    # per-core arrays
    COLS = int(sum(k_m[m] * 2 * m for m in range(1, maxm + 1)))  # class 0: none
    col_off = {}
    acc = 0
    for m in range(1, maxm + 1):
        col_off[m] = acc
        acc += int(k_m[m] * 2 * m)

    per_core = []
    for c in range(NCORES):
        idx = np.zeros((P, COLS), np.int32)
        # dummy slot: class 0 forced block guarantees padding
        n_real = len(class_lists[c][0]) if class_lists[c][0] is not None else 0
        t_d = n_real                            # first padded slot in class 0
        dummy_row = c * SHPAD + (j_off[0] + t_d // P) * P + (t_d % P)
        for m in range(1, maxm + 1):
            lst = class_lists[c][m]
            n = len(lst) if lst is not None else 0
            npad = int(k_m[m] * P)
            L = 2 * m
            em = np.full((npad, L), dummy_row, np.int64)
            if n:
                offs = segstart[lst]
                dg = deg_g[lst]                # exclude trailing self-loop edge
                u = np.arange(L)[None, :]
                take = offs[:, None] + np.minimum(u, np.maximum(dg[:, None] - 1, 0))
                vals = rowid[src_s[take]]
                mask = u < dg[:, None]
                em[:n] = np.where(mask, vals, dummy_row)
            em = em.reshape(int(k_m[m]), P, L).transpose(1, 0, 2).reshape(P, int(k_m[m]) * L)
            idx[:, col_off[m]:col_off[m] + int(k_m[m]) * L] = em

        # layout-order per-node values for this core
        vs = np.where(cores_of == np.int64(c))[0]
        q = J[vs] * P + Pp[vs]                 # shard row of each node
        dinv_q = np.zeros(SHPAD, np.float32)
        dinv_q[q] = dinv[vs]
        Aq = (0.9 * dinv_q * dinv_q).astype(np.float32)
        Cq = (0.9 * dinv_q).astype(np.float32)

        def expand(a):                         # [SHPAD] -> [P, K_TOT*D]
            M = a.reshape(K_TOT, P).T          # [P, K_TOT]
            return np.repeat(M[:, :, None], D, axis=2).reshape(P, K_TOT * D)

        A_e = expand(Aq).astype(bf16)
        C_e = expand(Cq).astype(bf16)
        dinv_e = expand(dinv_q).astype(bf16)

        xT = np.zeros((512, SHPAD), bf16)
        xT[:NFEAT, q] = x[vs].T.astype(bf16)

        per_core.append(dict(idx=idx, A=A_e, C=C_e, dinv=dinv_e, xT=xT))

    # weights (feature-major, zero-padded contraction dims)
    W0T = np.zeros((512, 256), bf16)
    W0T[:NFEAT] = W0.T.astype(bf16)
    W1T = W1.T.astype(bf16)                    # [256, 128]
    W2T = np.zeros((128, 64), bf16)
    W2T[:, :D] = W2.T.astype(bf16)             # [128, 40->64]
    b0t = b0.reshape(2, P).T.astype(np.float32)         # [128, 2]
    b1t = b1.reshape(1, P).T.astype(np.float32)         # [128, 1]
    b2t = np.zeros((P, 1), np.float32)
    b2t[:D, 0] = b2

    meta = dict(maxm=maxm, k_m=k_m, j_off=j_off, col_off=col_off, K_TOT=K_TOT,
                SHPAD=SHPAD, COLS=COLS, J=J, Pp=Pp, cores_of=cores_of)
    consts = dict(W0T=W0T, W1T=W1T, W2T=W2T, b0=b0t, b1=b1t, b2=b2t)
    return meta, per_core, consts


# --------------------------------------------------------------------------
# device program
# --------------------------------------------------------------------------
def _build(meta):
    import concourse.bass as bass
    import concourse.mybir as mybir
    import concourse.tile as tile_mod
    from concourse.masks import make_identity

    maxm = meta["maxm"]; k_m = meta["k_m"]; j_off = meta["j_off"]
    col_off = meta["col_off"]; K_TOT = meta["K_TOT"]; SHPAD = meta["SHPAD"]
    COLS = meta["COLS"]
    BF = mybir.dt.bfloat16
    F32 = mybir.dt.float32

    nc = bass.Bass(trn_type="TRN2", dynamic_dma_scratch_size=65536)
    xT_in = nc.declare_dram_parameter("xT", [512, SHPAD], BF, isOutput=False)
    idx_in = nc.declare_dram_parameter("idx", [P, COLS], mybir.dt.int32, isOutput=False)
    A_in = nc.declare_dram_parameter("A", [P, K_TOT * D], BF, isOutput=False)
    C_in = nc.declare_dram_parameter("C", [P, K_TOT * D], BF, isOutput=False)
    dinv_in = nc.declare_dram_parameter("dinv", [P, K_TOT * D], BF, isOutput=False)
    W0_in = nc.declare_dram_parameter("W0T", [512, 256], BF, isOutput=False)
    W1_in = nc.declare_dram_parameter("W1T", [256, 128], BF, isOutput=False)
    W2_in = nc.declare_dram_parameter("W2T", [128, 64], BF, isOutput=False)
    b0_in = nc.declare_dram_parameter("b0", [P, 2], F32, isOutput=False)
    b1_in = nc.declare_dram_parameter("b1", [P, 1], F32, isOutput=False)
    b2_in = nc.declare_dram_parameter("b2", [P, 1], F32, isOutput=False)
    out_ext = nc.declare_dram_parameter("out", [SHPAD, D], F32, isOutput=True)

    shard = [nc.dram_tensor(f"shard{i}", [SHPAD, D], BF) for i in range(2)]
    gath = [nc.dram_tensor(f"gath{i}", [NCORES * SHPAD, D], BF, addr_space="Shared")
            for i in range(2)]

    NB = SHPAD // 512

    with tile_mod.TileContext(nc) as tc:
        with tc.tile_pool(name="res", bufs=1) as res, \
             tc.tile_pool(name="mlp", bufs=2) as mlp, \
             tc.tile_pool(name="ps", bufs=2, space="PSUM") as ps, \
             tc.tile_pool(name="pst", bufs=2, space="PSUM") as pst, \
             tc.tile_pool(name="gp", bufs=2) as gp, \
             tc.tile_pool(name="rp", bufs=2) as rp:

            # ---- resident tensors ----
            idx = res.tile([P, COLS], mybir.dt.int32)
            nc.sync.dma_start(out=idx[:], in_=idx_in[:, :])
            A_t = res.tile([P, K_TOT * D], BF)
            C_t = res.tile([P, K_TOT * D], BF)
            dv_t = res.tile([P, K_TOT * D], BF)
            nc.sync.dma_start(out=A_t[:], in_=A_in[:, :])
            nc.sync.dma_start(out=C_t[:], in_=C_in[:, :])
            nc.sync.dma_start(out=dv_t[:], in_=dinv_in[:, :])
            w0 = res.tile([P, 4 * 256], BF)
            nc.sync.dma_start(out=w0[:].rearrange("p (k n) -> p k n", n=256),
                              in_=W0_in.ap().rearrange("(k p) n -> p k n", p=P))
            w1 = res.tile([P, 2 * 128], BF)
            nc.sync.dma_start(out=w1[:].rearrange("p (k n) -> p k n", n=128),
                              in_=W1_in.ap().rearrange("(k p) n -> p k n", p=P))
            w2 = res.tile([P, 64], BF)
            nc.sync.dma_start(out=w2[:], in_=W2_in[:, :])
            b0t = res.tile([P, 2], F32)
            b1t = res.tile([P, 1], F32)
            b2t = res.tile([P, 1], F32)
            nc.sync.dma_start(out=b0t[:], in_=b0_in[:, :])
            nc.sync.dma_start(out=b1t[:], in_=b1_in[:, :])
            nc.sync.dma_start(out=b2t[:], in_=b2_in[:, :])
            ident = res.tile([P, P], F32)
            make_identity(nc, ident[:])

            h0f = res.tile([P, K_TOT * D], F32)      # MLP output, node-major
            newtab = res.tile([P, K_TOT * D], BF)    # current scaled table
            B_t = res.tile([P, K_TOT * D], BF)       # 0.1*dinv*h0
            D_t = res.tile([P, K_TOT * D], BF)       # 0.1*h0

            # ---- MLP ----
            for b in range(NB):
                xt = mlp.tile([P, 4 * 512], BF, tag="xt")
                nc.sync.dma_start(
                    out=xt[:].rearrange("p (k n) -> p k n", n=512),
                    in_=xT_in.ap().rearrange("(k p) n -> p k n", p=P)[:, :, b * 512:(b + 1) * 512])
                h1t = mlp.tile([P, 2 * 512], BF, tag="h1")
                for oc in range(2):
                    pm = ps.tile([P, 512], F32, tag="pm")
                    for kc in range(4):
                        nc.tensor.matmul(
                            pm[:],
                            lhsT=w0[:, kc * 256 + oc * 128: kc * 256 + (oc + 1) * 128],
                            rhs=xt[:, kc * 512:(kc + 1) * 512],
                            start=(kc == 0), stop=(kc == 3))
                    nc.scalar.activation(h1t[:, oc * 512:(oc + 1) * 512], pm[:],
                                         mybir.ActivationFunctionType.Relu,
                                         bias=b0t[:, oc:oc + 1])
                pm2 = ps.tile([P, 512], F32, tag="pm")
                for kc in range(2):
                    nc.tensor.matmul(pm2[:], lhsT=w1[:, kc * 128:(kc + 1) * 128],
                                     rhs=h1t[:, kc * 512:(kc + 1) * 512],
                                     start=(kc == 0), stop=(kc == 1))
                h2t = mlp.tile([P, 512], BF, tag="h2")
                nc.scalar.activation(h2t[:], pm2[:], mybir.ActivationFunctionType.Relu,
                                     bias=b1t[:, 0:1])
                pm3 = ps.tile([P, 512], F32, tag="pm")
                nc.tensor.matmul(pm3[:64, :], lhsT=w2[:], rhs=h2t[:], start=True, stop=True)
                h3s = mlp.tile([P, 512], F32, tag="h3")
                nc.scalar.activation(h3s[:64, :], pm3[:64, :],
                                     mybir.ActivationFunctionType.Relu, bias=b2t[:64, 0:1])
                for i in range(4):
                    tr = pst.tile([P, P], F32, tag="tr")
                    nc.tensor.transpose(out=tr[:], in_=h3s[:, i * P:(i + 1) * P],
                                        identity=ident[:])
                    jcol = b * 4 + i
                    nc.vector.tensor_copy(out=h0f[:, jcol * D:(jcol + 1) * D],
                                          in_=tr[:, :D])

            # ---- propagation setup ----
            nc.vector.tensor_tensor(out=newtab[:], in0=dv_t[:], in1=h0f[:],
                                    op=mybir.AluOpType.mult)
            nc.vector.tensor_scalar_mul(B_t[:], newtab[:], 0.1)
            nc.vector.tensor_scalar_mul(D_t[:], h0f[:], 0.1)

            nc.sync.dma_start(out=shard[0].ap().rearrange("(j p) d -> p j d", p=P),
                              in_=newtab[:].rearrange("p (j d) -> p j d", d=D))
            nc.gpsimd.collective_compute(
                "AllGather", mybir.AluOpType.bypass,
                replica_groups=[list(range(NCORES))],
                ins=[shard[0].ap().opt()], outs=[gath[0].ap().opt()])

            # ---- K propagation steps ----
            for k in range(K):
                gbuf = gath[k % 2]
                last = (k == K - 1)
                for m in range(0, maxm + 1):
                    km = int(k_m[m])
                    if km == 0:
                        continue
                    L = 2 * m
                    JCH = max(1, min(km, 192 // L)) if m else min(km, 192)
                    j0 = 0
                    while j0 < km:
                        jc = min(JCH, km - j0)
                        jg = j_off[m] + j0                   # global j of chunk start
                        tabsl = newtab[:, jg * D:(jg + jc) * D].rearrange(
                            "p (j d) -> p j d", d=D)
                        if m == 0:
                            acc = tabsl                      # self term only
                        else:
                            cols0 = col_off[m] + j0 * L
                            g = gp.tile([P, JCH * L * D], BF, tag="g")
                            for t in range(jc * L):
                                nc.gpsimd.indirect_dma_start(
                                    out=g[:, t * D:(t + 1) * D],
                                    out_offset=None,
                                    in_=gbuf.ap(),
                                    in_offset=bass.IndirectOffsetOnAxis(
                                        ap=idx[:, cols0 + t:cols0 + t + 1], axis=0))
                            # pair-add: [p, jc*m, 2, D] -> part [p, jc*m, D]
                            part = rp.tile([P, JCH * m * D], BF, tag="part")
                            gv = g[:, :jc * L * D].rearrange("p (g two d) -> p g two d",
                                                             two=2, d=D)
                            pv = part[:, :jc * m * D].rearrange("p (g d) -> p g d", d=D)
                            nc.vector.tensor_tensor(out=pv, in0=gv[:, :, 0, :],
                                                    in1=gv[:, :, 1, :],
                                                    op=mybir.AluOpType.add)
                            # fold m partials -> acc in u=0 slice
                            p4 = part[:, :jc * m * D].rearrange("p (j u d) -> p j u d",
                                                                u=m, d=D)
                            for u in range(1, m):
                                nc.vector.tensor_tensor(out=p4[:, :, 0, :],
                                                        in0=p4[:, :, 0, :],
                                                        in1=p4[:, :, u, :],
                                                        op=mybir.AluOpType.add)
                            # add self-loop term (previous table values, resident)
                            nc.vector.tensor_tensor(out=p4[:, :, 0, :],
                                                    in0=p4[:, :, 0, :],
                                                    in1=tabsl,
                                                    op=mybir.AluOpType.add)
                            acc = p4[:, :, 0, :]             # [p, jc, D] stride m*D
                        asl = A_t[:, jg * D:(jg + jc) * D].rearrange(
                            "p (j d) -> p j d", d=D)
                        if last:
                            ob = rp.tile([P, JCH * D], F32, tag="ob")
                            obv = ob[:, :jc * D].rearrange("p (j d) -> p j d", d=D)
                            csl = C_t[:, jg * D:(jg + jc) * D].rearrange(
                                "p (j d) -> p j d", d=D)
                            dsl = D_t[:, jg * D:(jg + jc) * D].rearrange(
                                "p (j d) -> p j d", d=D)
                            nc.vector.tensor_tensor(out=obv, in0=acc, in1=csl,
                                                    op=mybir.AluOpType.mult)
                            nc.vector.tensor_tensor(out=obv, in0=obv, in1=dsl,
                                                    op=mybir.AluOpType.add)
                            nc.sync.dma_start(
                                out=out_ext.ap().rearrange("(j p) d -> p j d", p=P)[:, jg:jg + jc, :],
                                in_=obv)
                        else:
                            bsl = B_t[:, jg * D:(jg + jc) * D].rearrange(
                                "p (j d) -> p j d", d=D)
                            nc.vector.tensor_tensor(out=tabsl, in0=acc, in1=asl,
                                                    op=mybir.AluOpType.mult)
                            nc.vector.tensor_tensor(out=tabsl, in0=tabsl, in1=bsl,
                                                    op=mybir.AluOpType.add)
                        j0 += jc
                if not last:
                    sb = shard[(k + 1) % 2]
                    gb = gath[(k + 1) % 2]
                    nc.sync.dma_start(out=sb.ap().rearrange("(j p) d -> p j d", p=P),
                                      in_=newtab[:].rearrange("p (j d) -> p j d", d=D))
                    nc.gpsimd.collective_compute(
                        "AllGather", mybir.AluOpType.bypass,
                        replica_groups=[list(range(NCORES))],
                        ins=[sb.ap().opt()], outs=[gb.ap().opt()])
    return nc


# --------------------------------------------------------------------------
# entry point
# --------------------------------------------------------------------------
def kernel(x, edge_index, W0, b0, W1, b1, W2, b2, _trace=False):
    _install_wait_split()
    from concourse.bass_utils import run_bass_kernel_spmd

    x = np.asarray(x, np.float32)
    edge_index = np.asarray(edge_index, np.int64)
    meta, per_core, consts = _preprocess(np.asarray(edge_index), x,
                                         np.asarray(W0), np.asarray(b0),
                                         np.asarray(W1), np.asarray(b1),
                                         np.asarray(W2), np.asarray(b2))
    nc = _build(meta)
    in_maps = []
    for c in range(NCORES):
        pc = per_core[c]
        in_maps.append({
            "xT": pc["xT"], "idx": pc["idx"], "A": pc["A"], "C": pc["C"],
            "dinv": pc["dinv"], "W0T": consts["W0T"], "W1T": consts["W1T"],
            "W2T": consts["W2T"], "b0": consts["b0"], "b1": consts["b1"],
            "b2": consts["b2"],
        })
    res = run_bass_kernel_spmd(nc, in_maps, core_ids=list(range(NCORES)),
                               trace=_trace)
    J, Pp, cores_of = meta["J"], meta["Pp"], meta["cores_of"]
    out = np.zeros((N, D), np.float32)
    rows = J * P + Pp
    for c in range(NCORES):
        vs = np.where(cores_of == np.int64(c))[0]
        out[vs] = res.results[c]["out"][rows[vs]]
    kernel.last_exec_time_ns = res.exec_time_ns
    return out


# revision 10
# speedup vs baseline: 1.1767x; 1.1767x over previous
"""APPNP (MLP + K-step personalized-pagerank propagation) on 8 TRN2 NeuronCores.

Strategy:
  * Nodes are relabeled into a per-core "class layout": each core owns 12500
    destination nodes; per node the (self-loop-inclusive) degree is padded to
    a multiple of 2 and nodes are grouped into classes by padded degree.
  * norm = dinv[src]*dinv[dst] factorizes, so the propagated table holds
    h_scaled = dinv*h (bf16) and the per-step blend is two elementwise ops.
  * Per step: AllGather the bf16 table shards (DRAM collective), gather each
    edge-slot row with one-index-per-partition indirect DMAs (128 slots per
    call), pair-add + fold on the Vector engine (segment sums), blend, repeat.
  * The 3-layer MLP runs feature-major in bf16 on the TensorEngine with fp32
    PSUM accumulation; outputs are transposed back to node-major via PE.

All graph preprocessing (sorting, padding, index tables) is host-side numpy;
indices are step-invariant and stay resident in SBUF.
"""

import numpy as np
import ml_dtypes

N = 100000
E = 1600000
NFEAT = 500
NCLASS = 40
K = 10
ALPHA = 0.1
NCORES = 8
SH = N // NCORES          # real dsts per core
D = NCLASS
P = 128

bf16 = ml_dtypes.bfloat16

# --------------------------------------------------------------------------
# compat patch: this walrus build rejects >1 sync-wait per instruction.
# Hoist excess waits onto standalone EventSemaphore instructions.
# --------------------------------------------------------------------------
_PATCHED = False


def _install_wait_split():
    global _PATCHED
    if _PATCHED:
        return
    import orjson
    import concourse.bass as _bass

    _orig = _bass.Bass.to_json_bytes

    def _patched(self):
        j = orjson.loads(_orig(self))
        for func in j.get("functions", []):
            for blk in func.get("blocks", []):
                insts = blk.get("instructions")
                if not insts:
                    continue
                out = []
                for inst in insts:
                    si = inst.get("sync_info") or {}
                    waits = si.get("on_wait") or []
                    if len(waits) > 1:
                        for k2, w in enumerate(waits[:-1]):
                            out.append({
                                "debug": inst.get("debug", 0),
                                "engine": inst["engine"],
                                "ins": [],
                                "name": f"{inst['name']}_sw{k2}",
                                "opcode": "EventSemaphore",
                                "outs": [],
                                "sync_info": {"on_update": [], "on_wait": [w]},
                            })
                        si["on_wait"] = [waits[-1]]
                        inst["sync_info"] = si
                    out.append(inst)
                blk["instructions"] = out
        return orjson.dumps(j)

    _bass.Bass.to_json_bytes = _patched
    _PATCHED = True


# --------------------------------------------------------------------------
# host-side graph preprocessing
# --------------------------------------------------------------------------
def _preprocess(edge_index, x, W0, b0, W1, b1, W2, b2):
    src = np.concatenate([edge_index[0], np.arange(N, dtype=np.int64)])
    dst = np.concatenate([edge_index[1], np.arange(N, dtype=np.int64)])
    deg = np.bincount(dst, minlength=N).astype(np.int64)       # >= 1
    dinv = (1.0 / np.sqrt(deg.astype(np.float32))).astype(np.float32)

    order = np.argsort(dst, kind="stable")
    src_s = src[order].astype(np.int64)
    segstart = np.zeros(N + 1, np.int64)
    np.cumsum(deg, out=segstart[1:])

    deg_g = deg - 1                        # gather degree (self-loop folded on-chip)
    m_of = (deg_g + 1) // 2                # class id per node (0 = no gather)
    maxm = int(m_of.max())
    # merge sparse classes upward (block-roundup waste dominates small classes)
    cg = np.bincount(m_of, minlength=maxm + 1)
    kept = [m for m in range(1, maxm + 1) if cg[m] >= 1024]
    if not kept or kept[-1] != maxm:
        kept.append(maxm)
    remap = np.zeros(maxm + 1, np.int64)
    for m in range(1, maxm + 1):
        remap[m] = next((kk for kk in kept if kk >= m), maxm)
    m_of = np.where(m_of >= 1, remap[np.minimum(m_of, maxm)], 0)

    # class-balanced dst->core assignment: deal each class round-robin
    cores_of = np.zeros(N, np.int64)
    k_m = np.zeros(maxm + 1, np.int64)
    global_class = [None] * (maxm + 1)
    for m in range(0, maxm + 1):
        lst = np.where(m_of == m)[0]
        global_class[m] = lst
        cores_of[lst] = np.arange(len(lst)) % NCORES
        per_core_max = (len(lst) + NCORES - 1) // NCORES
        k_m[m] = int(np.ceil(per_core_max / P))
    k_m[0] += 1                                # forced all-dummy block (zero rows)
    while int(k_m.sum()) % 4 != 0:
        k_m[0] += 1
    K_TOT = int(k_m.sum())
    SHPAD = P * K_TOT
    j_off = np.zeros(maxm + 1, np.int64)
    acc = 0
    for m in range(0, maxm + 1):
        j_off[m] = acc
        acc += k_m[m]

    # assignment: J[v], Pp[v]
    J = np.zeros(N, np.int64)
    Pp = np.zeros(N, np.int64)
    class_lists = [[None] * (maxm + 1) for _ in range(NCORES)]
    for m in range(0, maxm + 1):
        lst = global_class[m]
        for c in range(NCORES):
            sub = lst[cores_of[lst] == np.int64(c)]
            class_lists[c][m] = sub
            t = np.arange(len(sub))
            J[sub] = j_off[m] + t // P
            Pp[sub] = t % P
    rowid = cores_of * SHPAD + J * P + Pp      # global table row per node

    # per-core arrays
    COLS = int(sum(k_m[m] * 2 * m for m in range(1, maxm + 1)))  # class 0: none
    col_off = {}
    acc = 0
    for m in range(1, maxm + 1):
        col_off[m] = acc
        acc += int(k_m[m] * 2 * m)

    per_core = []
    for c in range(NCORES):
        idx = np.zeros((P, COLS), np.int32)
        # dummy slot: class 0 forced block guarantees padding
        n_real = len(class_lists[c][0]) if class_lists[c][0] is not None else 0
        t_d = n_real                            # first padded slot in class 0
        dummy_row = c * SHPAD + (j_off[0] + t_d // P) * P + (t_d % P)
        for m in range(1, maxm + 1):
            lst = class_lists[c][m]
            n = len(lst) if lst is not None else 0
            npad = int(k_m[m] * P)
            L = 2 * m
            em = np.full((npad, L), dummy_row, np.int64)
            if n:
                offs = segstart[lst]
                dg = deg_g[lst]                # exclude trailing self-loop edge
                u = np.arange(L)[None, :]
                take = offs[:, None] + np.minimum(u, np.maximum(dg[:, None] - 1, 0))
                vals = rowid[src_s[take]]
                mask = u < dg[:, None]
                em[:n] = np.where(mask, vals, dummy_row)
            em = em.reshape(int(k_m[m]), P, L).transpose(1, 0, 2).reshape(P, int(k_m[m]) * L)
            idx[:, col_off[m]:col_off[m] + int(k_m[m]) * L] = em

        # layout-order per-node values for this core
        vs = np.where(cores_of == np.int64(c))[0]
        q = J[vs] * P + Pp[vs]                 # shard row of each node
        dinv_q = np.zeros(SHPAD, np.float32)
        dinv_q[q] = dinv[vs]
        Aq = (0.9 * dinv_q * dinv_q).astype(np.float32)
        Cq = (0.9 * dinv_q).astype(np.float32)

        def expand(a):                         # [SHPAD] -> [P, K_TOT*D]
            M = a.reshape(K_TOT, P).T          # [P, K_TOT]
            return np.repeat(M[:, :, None], D, axis=2).reshape(P, K_TOT * D)

        A_e = expand(Aq).astype(bf16)
        C_e = expand(Cq).astype(bf16)
        dinv_e = expand(dinv_q).astype(bf16)

        xT = np.zeros((512, SHPAD), bf16)
        xT[:NFEAT, q] = x[vs].T.astype(bf16)

        per_core.append(dict(idx=idx, A=A_e, C=C_e, dinv=dinv_e, xT=xT))

    # weights (feature-major, zero-padded contraction dims)
    W0T = np.zeros((512, 256), bf16)
    W0T[:NFEAT] = W0.T.astype(bf16)
    W1T = W1.T.astype(bf16)                    # [256, 128]
    W2T = np.zeros((128, 64), bf16)
    W2T[:, :D] = W2.T.astype(bf16)             # [128, 40->64]
    b0t = b0.reshape(2, P).T.astype(np.float32)         # [128, 2]
    b1t = b1.reshape(1, P).T.astype(np.float32)         # [128, 1]
    b2t = np.zeros((P, 1), np.float32)
    b2t[:D, 0] = b2

    meta = dict(maxm=maxm, k_m=k_m, j_off=j_off, col_off=col_off, K_TOT=K_TOT,
                SHPAD=SHPAD, COLS=COLS, J=J, Pp=Pp, cores_of=cores_of)
    consts = dict(W0T=W0T, W1T=W1T, W2T=W2T, b0=b0t, b1=b1t, b2=b2t)
    return meta, per_core, consts


# --------------------------------------------------------------------------
# device program
# --------------------------------------------------------------------------
def _build(meta):
    import concourse.bass as bass
    import concourse.mybir as mybir
    import concourse.tile as tile_mod
    from concourse.masks import make_identity

    maxm = meta["maxm"]; k_m = meta["k_m"]; j_off = meta["j_off"]
    col_off = meta["col_off"]; K_TOT = meta["K_TOT"]; SHPAD = meta["SHPAD"]
    COLS = meta["COLS"]
    BF = mybir.dt.bfloat16
    F32 = mybir.dt.float32

    nc = bass.Bass(trn_type="TRN2", dynamic_dma_scratch_size=65536)
    xT_in = nc.declare_dram_parameter("xT", [512, SHPAD], BF, isOutput=False)
    idx_in = nc.declare_dram_parameter("idx", [P, COLS], mybir.dt.int32, isOutput=False)
    A_in = nc.declare_dram_parameter("A", [P, K_TOT * D], BF, isOutput=False)
    C_in = nc.declare_dram_parameter("C", [P, K_TOT * D], BF, isOutput=False)
    dinv_in = nc.declare_dram_parameter("dinv", [P, K_TOT * D], BF, isOutput=False)
    W0_in = nc.declare_dram_parameter("W0T", [512, 256], BF, isOutput=False)
    W1_in = nc.declare_dram_parameter("W1T", [256, 128], BF, isOutput=False)
    W2_in = nc.declare_dram_parameter("W2T", [128, 64], BF, isOutput=False)
    b0_in = nc.declare_dram_parameter("b0", [P, 2], F32, isOutput=False)
    b1_in = nc.declare_dram_parameter("b1", [P, 1], F32, isOutput=False)
    b2_in = nc.declare_dram_parameter("b2", [P, 1], F32, isOutput=False)
    out_ext = nc.declare_dram_parameter("out", [SHPAD, D], F32, isOutput=True)

    shard = [nc.dram_tensor(f"shard{i}", [SHPAD, D], BF) for i in range(2)]
    gath = [nc.dram_tensor(f"gath{i}", [NCORES * SHPAD, D], BF, addr_space="Shared")
            for i in range(2)]

    NB = SHPAD // 512

    with tile_mod.TileContext(nc) as tc:
        with tc.tile_pool(name="res", bufs=1) as res, \
             tc.tile_pool(name="mlp", bufs=2) as mlp, \
             tc.tile_pool(name="ps", bufs=2, space="PSUM") as ps, \
             tc.tile_pool(name="pst", bufs=2, space="PSUM") as pst, \
             tc.tile_pool(name="gp", bufs=2) as gp, \
             tc.tile_pool(name="rp", bufs=2) as rp:

            # ---- resident tensors ----
            idx = res.tile([P, COLS], mybir.dt.int32)
            nc.sync.dma_start(out=idx[:], in_=idx_in[:, :])
            A_t = res.tile([P, K_TOT * D], BF)
            C_t = res.tile([P, K_TOT * D], BF)
            dv_t = res.tile([P, K_TOT * D], BF)
            nc.sync.dma_start(out=A_t[:], in_=A_in[:, :])
            nc.sync.dma_start(out=C_t[:], in_=C_in[:, :])
            nc.sync.dma_start(out=dv_t[:], in_=dinv_in[:, :])
            w0 = res.tile([P, 4 * 256], BF)
            nc.sync.dma_start(out=w0[:].rearrange("p (k n) -> p k n", n=256),
                              in_=W0_in.ap().rearrange("(k p) n -> p k n", p=P))
            w1 = res.tile([P, 2 * 128], BF)
            nc.sync.dma_start(out=w1[:].rearrange("p (k n) -> p k n", n=128),
                              in_=W1_in.ap().rearrange("(k p) n -> p k n", p=P))
            w2 = res.tile([P, 64], BF)
            nc.sync.dma_start(out=w2[:], in_=W2_in[:, :])
            b0t = res.tile([P, 2], F32)
            b1t = res.tile([P, 1], F32)
            b2t = res.tile([P, 1], F32)
            nc.sync.dma_start(out=b0t[:], in_=b0_in[:, :])
            nc.sync.dma_start(out=b1t[:], in_=b1_in[:, :])
            nc.sync.dma_start(out=b2t[:], in_=b2_in[:, :])
            ident = res.tile([P, P], F32)
            make_identity(nc, ident[:])

            h0f = res.tile([P, K_TOT * D], F32)      # MLP output, node-major
            newtab = res.tile([P, K_TOT * D], BF)    # current scaled table
            B_t = res.tile([P, K_TOT * D], BF)       # 0.1*dinv*h0
            D_t = res.tile([P, K_TOT * D], BF)       # 0.1*h0

            # ---- MLP ----
            for b in range(NB):
                xt = mlp.tile([P, 4 * 512], BF, tag="xt")
                nc.sync.dma_start(
                    out=xt[:].rearrange("p (k n) -> p k n", n=512),
                    in_=xT_in.ap().rearrange("(k p) n -> p k n", p=P)[:, :, b * 512:(b + 1) * 512])
                h1t = mlp.tile([P, 2 * 512], BF, tag="h1")
                for oc in range(2):
                    pm = ps.tile([P, 512], F32, tag="pm")
                    for kc in range(4):
                        nc.tensor.matmul(
                            pm[:],
                            lhsT=w0[:, kc * 256 + oc * 128: kc * 256 + (oc + 1) * 128],
                            rhs=xt[:, kc * 512:(kc + 1) * 512],
                            start=(kc == 0), stop=(kc == 3))
                    nc.scalar.activation(h1t[:, oc * 512:(oc + 1) * 512], pm[:],
                                         mybir.ActivationFunctionType.Relu,
                                         bias=b0t[:, oc:oc + 1])
                pm2 = ps.tile([P, 512], F32, tag="pm")
                for kc in range(2):
                    nc.tensor.matmul(pm2[:], lhsT=w1[:, kc * 128:(kc + 1) * 128],
                                     rhs=h1t[:, kc * 512:(kc + 1) * 512],
                                     start=(kc == 0), stop=(kc == 1))
                h2t = mlp.tile([P, 512], BF, tag="h2")
                nc.scalar.activation(h2t[:], pm2[:], mybir.ActivationFunctionType.Relu,
                                     bias=b1t[:, 0:1])
                pm3 = ps.tile([P, 512], F32, tag="pm")
                nc.tensor.matmul(pm3[:64, :], lhsT=w2[:], rhs=h2t[:], start=True, stop=True)
                h3s = mlp.tile([P, 512], F32, tag="h3")
                nc.scalar.activation(h3s[:64, :], pm3[:64, :],
                                     mybir.ActivationFunctionType.Relu, bias=b2t[:64, 0:1])
                for i in range(4):
                    tr = pst.tile([P, P], F32, tag="tr")
                    nc.tensor.transpose(out=tr[:], in_=h3s[:, i * P:(i + 1) * P],
                                        identity=ident[:])
                    jcol = b * 4 + i
                    nc.vector.tensor_copy(out=h0f[:, jcol * D:(jcol + 1) * D],
                                          in_=tr[:, :D])

            # ---- propagation setup ----
            nc.vector.tensor_tensor(out=newtab[:], in0=dv_t[:], in1=h0f[:],
                                    op=mybir.AluOpType.mult)
            nc.vector.tensor_scalar_mul(B_t[:], newtab[:], 0.1)
            nc.vector.tensor_scalar_mul(D_t[:], h0f[:], 0.1)

            nc.sync.dma_start(out=shard[0].ap().rearrange("(j p) d -> p j d", p=P),
                              in_=newtab[:].rearrange("p (j d) -> p j d", d=D))
            nc.gpsimd.collective_compute(
                "AllGather", mybir.AluOpType.bypass,
                replica_groups=[list(range(NCORES))],
                ins=[shard[0].ap().opt()], outs=[gath[0].ap().opt()])

            # ---- K propagation steps ----
            for k in range(K):
                gbuf = gath[k % 2]
                last = (k == K - 1)
                for m in range(0, maxm + 1):
                    km = int(k_m[m])
                    if km == 0:
                        continue
                    L = 2 * m
                    JCH = max(1, min(km, 192 // L)) if m else min(km, 192)
                    j0 = 0
                    while j0 < km:
                        jc = min(JCH, km - j0)
                        jg = j_off[m] + j0                   # global j of chunk start
                        tabsl = newtab[:, jg * D:(jg + jc) * D].rearrange(
                            "p (j d) -> p j d", d=D)
                        if m == 0:
                            acc = tabsl                      # self term only
                        else:
                            cols0 = col_off[m] + j0 * L
                            g = gp.tile([P, JCH * L * D], BF, tag="g")
                            for t in range(jc * L):
                                nc.gpsimd.indirect_dma_start(
                                    out=g[:, t * D:(t + 1) * D],
                                    out_offset=None,
                                    in_=gbuf.ap(),
                                    in_offset=bass.IndirectOffsetOnAxis(
                                        ap=idx[:, cols0 + t:cols0 + t + 1], axis=0))
                            # pair-add: [p, jc*m, 2, D] -> part [p, jc*m, D]
                            part = rp.tile([P, JCH * m * D], BF, tag="part")
                            gv = g[:, :jc * L * D].rearrange("p (g two d) -> p g two d",
                                                             two=2, d=D)
                            pv = part[:, :jc * m * D].rearrange("p (g d) -> p g d", d=D)
                            nc.vector.tensor_tensor(out=pv, in0=gv[:, :, 0, :],
                                                    in1=gv[:, :, 1, :],
                                                    op=mybir.AluOpType.add)
                            # fold m partials -> acc in u=0 slice
                            p4 = part[:, :jc * m * D].rearrange("p (j u d) -> p j u d",
                                                                u=m, d=D)
                            for u in range(1, m):
                                nc.vector.tensor_tensor(out=p4[:, :, 0, :],
                                                        in0=p4[:, :, 0, :],
                                                        in1=p4[:, :, u, :],
                                                        op=mybir.AluOpType.add)
                            # add self-loop term (previous table values, resident)
                            nc.vector.tensor_tensor(out=p4[:, :, 0, :],
                                                    in0=p4[:, :, 0, :],
                                                    in1=tabsl,
                                                    op=mybir.AluOpType.add)
                            acc = p4[:, :, 0, :]             # [p, jc, D] stride m*D
                        asl = A_t[:, jg * D:(jg + jc) * D].rearrange(
                            "p (j d) -> p j d", d=D)
                        if last:
                            ob = rp.tile([P, JCH * D], F32, tag="ob")
                            obv = ob[:, :jc * D].rearrange("p (j d) -> p j d", d=D)
                            csl = C_t[:, jg * D:(jg + jc) * D].rearrange(
                                "p (j d) -> p j d", d=D)
                            dsl = D_t[:, jg * D:(jg + jc) * D].rearrange(
                                "p (j d) -> p j d", d=D)
                            nc.vector.tensor_tensor(out=obv, in0=acc, in1=csl,
                                                    op=mybir.AluOpType.mult)
                            nc.vector.tensor_tensor(out=obv, in0=obv, in1=dsl,
                                                    op=mybir.AluOpType.add)
                            nc.sync.dma_start(
                                out=out_ext.ap().rearrange("(j p) d -> p j d", p=P)[:, jg:jg + jc, :],
                                in_=obv)
                        else:
                            bsl = B_t[:, jg * D:(jg + jc) * D].rearrange(
                                "p (j d) -> p j d", d=D)
                            nc.vector.tensor_tensor(out=tabsl, in0=acc, in1=asl,
                                                    op=mybir.AluOpType.mult)
                            nc.vector.tensor_tensor(out=tabsl, in0=tabsl, in1=bsl,
                                                    op=mybir.AluOpType.add)
                        j0 += jc
                if not last:
                    sb = shard[(k + 1) % 2]
                    gb = gath[(k + 1) % 2]
                    nc.sync.dma_start(out=sb.ap().rearrange("(j p) d -> p j d", p=P),
                                      in_=newtab[:].rearrange("p (j d) -> p j d", d=D))
                    nc.gpsimd.collective_compute(
                        "AllGather", mybir.AluOpType.bypass,
                        replica_groups=[list(range(NCORES))],
                        ins=[sb.ap().opt()], outs=[gb.ap().opt()])
    return nc


# --------------------------------------------------------------------------
# entry point
# --------------------------------------------------------------------------
def kernel(x, edge_index, W0, b0, W1, b1, W2, b2, _trace=False):
    _install_wait_split()
    from concourse.bass_utils import run_bass_kernel_spmd

    x = np.asarray(x, np.float32)
    edge_index = np.asarray(edge_index, np.int64)
    meta, per_core, consts = _preprocess(np.asarray(edge_index), x,
                                         np.asarray(W0), np.asarray(b0),
                                         np.asarray(W1), np.asarray(b1),
                                         np.asarray(W2), np.asarray(b2))
    nc = _build(meta)
    in_maps = []
    for c in range(NCORES):
        pc = per_core[c]
        in_maps.append({
            "xT": pc["xT"], "idx": pc["idx"], "A": pc["A"], "C": pc["C"],
            "dinv": pc["dinv"], "W0T": consts["W0T"], "W1T": consts["W1T"],
            "W2T": consts["W2T"], "b0": consts["b0"], "b1": consts["b1"],
            "b2": consts["b2"],
        })
    res = run_bass_kernel_spmd(nc, in_maps, core_ids=list(range(NCORES)),
                               trace=_trace)
    J, Pp, cores_of = meta["J"], meta["Pp"], meta["cores_of"]
    out = np.zeros((N, D), np.float32)
    rows = J * P + Pp
    for c in range(NCORES):
        vs = np.where(cores_of == np.int64(c))[0]
        out[vs] = res.results[c]["out"][rows[vs]]
    kernel.last_exec_time_ns = res.exec_time_ns
    return out


# revision 11
# speedup vs baseline: 1.1838x; 1.0060x over previous
"""APPNP (MLP + K-step personalized-pagerank propagation) on 8 TRN2 NeuronCores.

Strategy:
  * Nodes are relabeled into a per-core "class layout": each core owns 12500
    destination nodes; per node the (self-loop-inclusive) degree is padded to
    a multiple of 2 and nodes are grouped into classes by padded degree.
  * norm = dinv[src]*dinv[dst] factorizes, so the propagated table holds
    h_scaled = dinv*h (bf16) and the per-step blend is two elementwise ops.
  * Per step: AllGather the bf16 table shards (DRAM collective), gather each
    edge-slot row with one-index-per-partition indirect DMAs (128 slots per
    call), pair-add + fold on the Vector engine (segment sums), blend, repeat.
  * The 3-layer MLP runs feature-major in bf16 on the TensorEngine with fp32
    PSUM accumulation; outputs are transposed back to node-major via PE.

All graph preprocessing (sorting, padding, index tables) is host-side numpy;
indices are step-invariant and stay resident in SBUF.
"""

import numpy as np
import ml_dtypes

N = 100000
E = 1600000
NFEAT = 500
NCLASS = 40
K = 10
ALPHA = 0.1
NCORES = 8
SH = N // NCORES          # real dsts per core
D = NCLASS
P = 128

bf16 = ml_dtypes.bfloat16

# --------------------------------------------------------------------------
# compat patch: this walrus build rejects >1 sync-wait per instruction.
# Hoist excess waits onto standalone EventSemaphore instructions.
# --------------------------------------------------------------------------
_PATCHED = False


def _install_wait_split():
    global _PATCHED
    if _PATCHED:
        return
    import orjson
    import concourse.bass as _bass

    _orig = _bass.Bass.to_json_bytes

    def _patched(self):
        j = orjson.loads(_orig(self))
        for func in j.get("functions", []):
            for blk in func.get("blocks", []):
                insts = blk.get("instructions")
                if not insts:
                    continue
                out = []
                for inst in insts:
                    si = inst.get("sync_info") or {}
                    waits = si.get("on_wait") or []
                    if len(waits) > 1:
                        for k2, w in enumerate(waits[:-1]):
                            out.append({
                                "debug": inst.get("debug", 0),
                                "engine": inst["engine"],
                                "ins": [],
                                "name": f"{inst['name']}_sw{k2}",
                                "opcode": "EventSemaphore",
                                "outs": [],
                                "sync_info": {"on_update": [], "on_wait": [w]},
                            })
                        si["on_wait"] = [waits[-1]]
                        inst["sync_info"] = si
                    out.append(inst)
                blk["instructions"] = out
        return orjson.dumps(j)

    _bass.Bass.to_json_bytes = _patched
    _PATCHED = True


# --------------------------------------------------------------------------
# host-side graph preprocessing
# --------------------------------------------------------------------------
def _preprocess(edge_index, x, W0, b0, W1, b1, W2, b2):
    src = np.concatenate([edge_index[0], np.arange(N, dtype=np.int64)])
    dst = np.concatenate([edge_index[1], np.arange(N, dtype=np.int64)])
    deg = np.bincount(dst, minlength=N).astype(np.int64)       # >= 1
    dinv = (1.0 / np.sqrt(deg.astype(np.float32))).astype(np.float32)

    order = np.argsort(dst, kind="stable")
    src_s = src[order].astype(np.int64)
    segstart = np.zeros(N + 1, np.int64)
    np.cumsum(deg, out=segstart[1:])

    deg_g = deg - 1                        # gather degree (self-loop folded on-chip)
    m_of = (deg_g + 1) // 2                # class id per node (0 = no gather)
    maxm = int(m_of.max())
    # merge sparse classes upward (block-roundup waste dominates small classes)
    cg = np.bincount(m_of, minlength=maxm + 1)
    kept = [m for m in range(1, maxm + 1) if cg[m] >= 1024]
    if not kept or kept[-1] != maxm:
        kept.append(maxm)
    remap = np.zeros(maxm + 1, np.int64)
    for m in range(1, maxm + 1):
        remap[m] = next((kk for kk in kept if kk >= m), maxm)
    m_of = np.where(m_of >= 1, remap[np.minimum(m_of, maxm)], 0)

    # class-balanced dst->core assignment: deal each class round-robin
    cores_of = np.zeros(N, np.int64)
    k_m = np.zeros(maxm + 1, np.int64)
    global_class = [None] * (maxm + 1)
    for m in range(0, maxm + 1):
        lst = np.where(m_of == m)[0]
        global_class[m] = lst
        cores_of[lst] = np.arange(len(lst)) % NCORES
        per_core_max = (len(lst) + NCORES - 1) // NCORES
        k_m[m] = int(np.ceil(per_core_max / P))
    k_m[0] += 1                                # forced all-dummy block (zero rows)
    while int(k_m.sum()) % 4 != 0:
        k_m[0] += 1
    K_TOT = int(k_m.sum())
    SHPAD = P * K_TOT
    j_off = np.zeros(maxm + 1, np.int64)
    acc = 0
    for m in range(0, maxm + 1):
        j_off[m] = acc
        acc += k_m[m]

    # assignment: J[v], Pp[v]
    J = np.zeros(N, np.int64)
    Pp = np.zeros(N, np.int64)
    class_lists = [[None] * (maxm + 1) for _ in range(NCORES)]
    for m in range(0, maxm + 1):
        lst = global_class[m]
        for c in range(NCORES):
            sub = lst[cores_of[lst] == np.int64(c)]
            class_lists[c][m] = sub
            t = np.arange(len(sub))
            J[sub] = j_off[m] + t // P
            Pp[sub] = t % P
    rowid = cores_of * SHPAD + J * P + Pp      # global table row per node

    # per-core arrays
    COLS = int(sum(k_m[m] * 2 * m for m in range(1, maxm + 1)))  # class 0: none
    col_off = {}
    acc = 0
    for m in range(1, maxm + 1):
        col_off[m] = acc
        acc += int(k_m[m] * 2 * m)

    per_core = []
    for c in range(NCORES):
        idx = np.zeros((P, COLS), np.int32)
        # dummy slot: class 0 forced block guarantees padding
        n_real = len(class_lists[c][0]) if class_lists[c][0] is not None else 0
        t_d = n_real                            # first padded slot in class 0
        dummy_row = c * SHPAD + (j_off[0] + t_d // P) * P + (t_d % P)
        for m in range(1, maxm + 1):
            lst = class_lists[c][m]
            n = len(lst) if lst is not None else 0
            npad = int(k_m[m] * P)
            L = 2 * m
            em = np.full((npad, L), dummy_row, np.int64)
            if n:
                offs = segstart[lst]
                dg = deg_g[lst]                # exclude trailing self-loop edge
                u = np.arange(L)[None, :]
                take = offs[:, None] + np.minimum(u, np.maximum(dg[:, None] - 1, 0))
                vals = rowid[src_s[take]]
                mask = u < dg[:, None]
                em[:n] = np.where(mask, vals, dummy_row)
            em = em.reshape(int(k_m[m]), P, L).transpose(1, 0, 2).reshape(P, int(k_m[m]) * L)
            idx[:, col_off[m]:col_off[m] + int(k_m[m]) * L] = em

        # layout-order per-node values for this core
        vs = np.where(cores_of == np.int64(c))[0]
        q = J[vs] * P + Pp[vs]                 # shard row of each node
        dinv_q = np.zeros(SHPAD, np.float32)
        dinv_q[q] = dinv[vs]
        Aq = (0.9 * dinv_q * dinv_q).astype(np.float32)
        Cq = (0.9 * dinv_q).astype(np.float32)

        def expand(a):                         # [SHPAD] -> [P, K_TOT*D]
            M = a.reshape(K_TOT, P).T          # [P, K_TOT]
            return np.repeat(M[:, :, None], D, axis=2).reshape(P, K_TOT * D)

        A_e = expand(Aq).astype(bf16)
        C_e = expand(Cq).astype(bf16)
        dinv_e = expand(dinv_q).astype(bf16)

        xT = np.zeros((512, SHPAD), bf16)
        xT[:NFEAT, q] = x[vs].T.astype(bf16)

        per_core.append(dict(idx=idx, A=A_e, C=C_e, dinv=dinv_e, xT=xT))

    # weights (feature-major, zero-padded contraction dims)
    W0T = np.zeros((512, 256), bf16)
    W0T[:NFEAT] = W0.T.astype(bf16)
    W1T = W1.T.astype(bf16)                    # [256, 128]
    W2T = np.zeros((128, 64), bf16)
    W2T[:, :D] = W2.T.astype(bf16)             # [128, 40->64]
    b0t = b0.reshape(2, P).T.astype(np.float32)         # [128, 2]
    b1t = b1.reshape(1, P).T.astype(np.float32)         # [128, 1]
    b2t = np.zeros((P, 1), np.float32)
    b2t[:D, 0] = b2

    meta = dict(maxm=maxm, k_m=k_m, j_off=j_off, col_off=col_off, K_TOT=K_TOT,
                SHPAD=SHPAD, COLS=COLS, J=J, Pp=Pp, cores_of=cores_of)
    consts = dict(W0T=W0T, W1T=W1T, W2T=W2T, b0=b0t, b1=b1t, b2=b2t)
    return meta, per_core, consts


# --------------------------------------------------------------------------
# device program
# --------------------------------------------------------------------------
def _build(meta):
    import concourse.bass as bass
    import concourse.mybir as mybir
    import concourse.tile as tile_mod
    from concourse.masks import make_identity

    maxm = meta["maxm"]; k_m = meta["k_m"]; j_off = meta["j_off"]
    col_off = meta["col_off"]; K_TOT = meta["K_TOT"]; SHPAD = meta["SHPAD"]
    COLS = meta["COLS"]
    BF = mybir.dt.bfloat16
    F32 = mybir.dt.float32

    nc = bass.Bass(trn_type="TRN2", dynamic_dma_scratch_size=65536)
    xT_in = nc.declare_dram_parameter("xT", [512, SHPAD], BF, isOutput=False)
    idx_in = nc.declare_dram_parameter("idx", [P, COLS], mybir.dt.int32, isOutput=False)
    A_in = nc.declare_dram_parameter("A", [P, K_TOT * D], BF, isOutput=False)
    C_in = nc.declare_dram_parameter("C", [P, K_TOT * D], BF, isOutput=False)
    dinv_in = nc.declare_dram_parameter("dinv", [P, K_TOT * D], BF, isOutput=False)
    W0_in = nc.declare_dram_parameter("W0T", [512, 256], BF, isOutput=False)
    W1_in = nc.declare_dram_parameter("W1T", [256, 128], BF, isOutput=False)
    W2_in = nc.declare_dram_parameter("W2T", [128, 64], BF, isOutput=False)
    b0_in = nc.declare_dram_parameter("b0", [P, 2], F32, isOutput=False)
    b1_in = nc.declare_dram_parameter("b1", [P, 1], F32, isOutput=False)
    b2_in = nc.declare_dram_parameter("b2", [P, 1], F32, isOutput=False)
    out_ext = nc.declare_dram_parameter("out", [SHPAD, D], F32, isOutput=True)

    shard = [nc.dram_tensor(f"shard{i}", [SHPAD, D], BF) for i in range(2)]
    gath = [nc.dram_tensor(f"gath{i}", [NCORES * SHPAD, D], BF, addr_space="Shared")
            for i in range(2)]

    NB = SHPAD // 512

    with tile_mod.TileContext(nc) as tc:
        with tc.tile_pool(name="res", bufs=1) as res, \
             tc.tile_pool(name="mlp", bufs=2) as mlp, \
             tc.tile_pool(name="ps", bufs=2, space="PSUM") as ps, \
             tc.tile_pool(name="pst", bufs=2, space="PSUM") as pst, \
             tc.tile_pool(name="gp", bufs=2) as gp, \
             tc.tile_pool(name="rp", bufs=2) as rp:

            # ---- resident tensors ----
            idx = res.tile([P, COLS], mybir.dt.int32)
            nc.sync.dma_start(out=idx[:], in_=idx_in[:, :])
            A_t = res.tile([P, K_TOT * D], BF)
            C_t = res.tile([P, K_TOT * D], BF)
            dv_t = res.tile([P, K_TOT * D], BF)
            nc.sync.dma_start(out=A_t[:], in_=A_in[:, :])
            nc.sync.dma_start(out=C_t[:], in_=C_in[:, :])
            nc.sync.dma_start(out=dv_t[:], in_=dinv_in[:, :])
            w0 = res.tile([P, 4 * 256], BF)
            nc.sync.dma_start(out=w0[:].rearrange("p (k n) -> p k n", n=256),
                              in_=W0_in.ap().rearrange("(k p) n -> p k n", p=P))
            w1 = res.tile([P, 2 * 128], BF)
            nc.sync.dma_start(out=w1[:].rearrange("p (k n) -> p k n", n=128),
                              in_=W1_in.ap().rearrange("(k p) n -> p k n", p=P))
            w2 = res.tile([P, 64], BF)
            nc.sync.dma_start(out=w2[:], in_=W2_in[:, :])
            b0t = res.tile([P, 2], F32)
            b1t = res.tile([P, 1], F32)
            b2t = res.tile([P, 1], F32)
            nc.sync.dma_start(out=b0t[:], in_=b0_in[:, :])
            nc.sync.dma_start(out=b1t[:], in_=b1_in[:, :])
            nc.sync.dma_start(out=b2t[:], in_=b2_in[:, :])
            ident = res.tile([P, P], F32)
            make_identity(nc, ident[:])

            h0f = res.tile([P, K_TOT * D], F32)      # MLP output, node-major
            newtab = res.tile([P, K_TOT * D], BF)    # current scaled table
            B_t = res.tile([P, K_TOT * D], BF)       # 0.1*dinv*h0
            D_t = res.tile([P, K_TOT * D], BF)       # 0.1*h0

            # ---- MLP ----
            for b in range(NB):
                xt = mlp.tile([P, 4 * 512], BF, tag="xt")
                nc.sync.dma_start(
                    out=xt[:].rearrange("p (k n) -> p k n", n=512),
                    in_=xT_in.ap().rearrange("(k p) n -> p k n", p=P)[:, :, b * 512:(b + 1) * 512])
                h1t = mlp.tile([P, 2 * 512], BF, tag="h1")
                for oc in range(2):
                    pm = ps.tile([P, 512], F32, tag="pm")
                    for kc in range(4):
                        nc.tensor.matmul(
                            pm[:],
                            lhsT=w0[:, kc * 256 + oc * 128: kc * 256 + (oc + 1) * 128],
                            rhs=xt[:, kc * 512:(kc + 1) * 512],
                            start=(kc == 0), stop=(kc == 3))
                    nc.scalar.activation(h1t[:, oc * 512:(oc + 1) * 512], pm[:],
                                         mybir.ActivationFunctionType.Relu,
                                         bias=b0t[:, oc:oc + 1])
                pm2 = ps.tile([P, 512], F32, tag="pm")
                for kc in range(2):
                    nc.tensor.matmul(pm2[:], lhsT=w1[:, kc * 128:(kc + 1) * 128],
                                     rhs=h1t[:, kc * 512:(kc + 1) * 512],
                                     start=(kc == 0), stop=(kc == 1))
                h2t = mlp.tile([P, 512], BF, tag="h2")
                nc.scalar.activation(h2t[:], pm2[:], mybir.ActivationFunctionType.Relu,
                                     bias=b1t[:, 0:1])
                pm3 = ps.tile([P, 512], F32, tag="pm")
                nc.tensor.matmul(pm3[:64, :], lhsT=w2[:], rhs=h2t[:], start=True, stop=True)
                h3s = mlp.tile([P, 512], F32, tag="h3")
                nc.scalar.activation(h3s[:64, :], pm3[:64, :],
                                     mybir.ActivationFunctionType.Relu, bias=b2t[:64, 0:1])
                for i in range(4):
                    tr = pst.tile([P, P], F32, tag="tr")
                    nc.tensor.transpose(out=tr[:], in_=h3s[:, i * P:(i + 1) * P],
                                        identity=ident[:])
                    jcol = b * 4 + i
                    nc.vector.tensor_copy(out=h0f[:, jcol * D:(jcol + 1) * D],
                                          in_=tr[:, :D])

            # ---- propagation setup ----
            nc.vector.tensor_tensor(out=newtab[:], in0=dv_t[:], in1=h0f[:],
                                    op=mybir.AluOpType.mult)
            nc.vector.tensor_scalar_mul(B_t[:], newtab[:], 0.1)
            nc.vector.tensor_scalar_mul(D_t[:], h0f[:], 0.1)

            nc.sync.dma_start(out=shard[0].ap().rearrange("(j p) d -> p j d", p=P),
                              in_=newtab[:].rearrange("p (j d) -> p j d", d=D))
            nc.gpsimd.collective_compute(
                "AllGather", mybir.AluOpType.bypass,
                replica_groups=[list(range(NCORES))],
                ins=[shard[0].ap().opt()], outs=[gath[0].ap().opt()])

            # ---- K propagation steps ----
            for k in range(K):
                gbuf = gath[k % 2]
                last = (k == K - 1)
                sb = shard[(k + 1) % 2]
                gb = gath[(k + 1) % 2]
                for m in range(0, maxm + 1):
                    km = int(k_m[m])
                    if km == 0:
                        continue
                    L = 2 * m
                    JCH = max(1, min(km, 192 // L)) if m else min(km, 192)
                    j0 = 0
                    while j0 < km:
                        jc = min(JCH, km - j0)
                        jg = j_off[m] + j0                   # global j of chunk start
                        tabsl = newtab[:, jg * D:(jg + jc) * D].rearrange(
                            "p (j d) -> p j d", d=D)
                        if m == 0:
                            acc = tabsl                      # self term only
                        else:
                            cols0 = col_off[m] + j0 * L
                            g = gp.tile([P, JCH * L * D], BF, tag="g")
                            for t in range(jc * L):
                                nc.gpsimd.indirect_dma_start(
                                    out=g[:, t * D:(t + 1) * D],
                                    out_offset=None,
                                    in_=gbuf.ap(),
                                    in_offset=bass.IndirectOffsetOnAxis(
                                        ap=idx[:, cols0 + t:cols0 + t + 1], axis=0))
                            # pair-add: [p, jc*m, 2, D] -> part [p, jc*m, D]
                            part = rp.tile([P, JCH * m * D], BF, tag="part")
                            gv = g[:, :jc * L * D].rearrange("p (g two d) -> p g two d",
                                                             two=2, d=D)
                            pv = part[:, :jc * m * D].rearrange("p (g d) -> p g d", d=D)
                            nc.vector.tensor_tensor(out=pv, in0=gv[:, :, 0, :],
                                                    in1=gv[:, :, 1, :],
                                                    op=mybir.AluOpType.add)
                            # fold m partials -> acc in u=0 slice
                            p4 = part[:, :jc * m * D].rearrange("p (j u d) -> p j u d",
                                                                u=m, d=D)
                            for u in range(1, m):
                                nc.vector.tensor_tensor(out=p4[:, :, 0, :],
                                                        in0=p4[:, :, 0, :],
                                                        in1=p4[:, :, u, :],
                                                        op=mybir.AluOpType.add)
                            # add self-loop term (previous table values, resident)
                            nc.vector.tensor_tensor(out=p4[:, :, 0, :],
                                                    in0=p4[:, :, 0, :],
                                                    in1=tabsl,
                                                    op=mybir.AluOpType.add)
                            acc = p4[:, :, 0, :]             # [p, jc, D] stride m*D
                        asl = A_t[:, jg * D:(jg + jc) * D].rearrange(
                            "p (j d) -> p j d", d=D)
                        if last:
                            ob = rp.tile([P, JCH * D], F32, tag="ob")
                            obv = ob[:, :jc * D].rearrange("p (j d) -> p j d", d=D)
                            csl = C_t[:, jg * D:(jg + jc) * D].rearrange(
                                "p (j d) -> p j d", d=D)
                            dsl = D_t[:, jg * D:(jg + jc) * D].rearrange(
                                "p (j d) -> p j d", d=D)
                            nc.vector.tensor_tensor(out=obv, in0=acc, in1=csl,
                                                    op=mybir.AluOpType.mult)
                            nc.vector.tensor_tensor(out=obv, in0=obv, in1=dsl,
                                                    op=mybir.AluOpType.add)
                            nc.sync.dma_start(
                                out=out_ext.ap().rearrange("(j p) d -> p j d", p=P)[:, jg:jg + jc, :],
                                in_=obv)
                        else:
                            bsl = B_t[:, jg * D:(jg + jc) * D].rearrange(
                                "p (j d) -> p j d", d=D)
                            nc.vector.tensor_tensor(out=tabsl, in0=acc, in1=asl,
                                                    op=mybir.AluOpType.mult)
                            nc.vector.tensor_tensor(out=tabsl, in0=tabsl, in1=bsl,
                                                    op=mybir.AluOpType.add)
                            nc.sync.dma_start(
                                out=sb.ap().rearrange("(j p) d -> p j d", p=P)[:, jg:jg + jc, :],
                                in_=tabsl)
                        j0 += jc
                if not last:
                    nc.gpsimd.collective_compute(
                        "AllGather", mybir.AluOpType.bypass,
                        replica_groups=[list(range(NCORES))],
                        ins=[sb.ap().opt()], outs=[gb.ap().opt()])
    return nc


# --------------------------------------------------------------------------
# entry point
# --------------------------------------------------------------------------
def kernel(x, edge_index, W0, b0, W1, b1, W2, b2, _trace=False):
    _install_wait_split()
    from concourse.bass_utils import run_bass_kernel_spmd

    x = np.asarray(x, np.float32)
    edge_index = np.asarray(edge_index, np.int64)
    meta, per_core, consts = _preprocess(np.asarray(edge_index), x,
                                         np.asarray(W0), np.asarray(b0),
                                         np.asarray(W1), np.asarray(b1),
                                         np.asarray(W2), np.asarray(b2))
    nc = _build(meta)
    in_maps = []
    for c in range(NCORES):
        pc = per_core[c]
        in_maps.append({
            "xT": pc["xT"], "idx": pc["idx"], "A": pc["A"], "C": pc["C"],
            "dinv": pc["dinv"], "W0T": consts["W0T"], "W1T": consts["W1T"],
            "W2T": consts["W2T"], "b0": consts["b0"], "b1": consts["b1"],
            "b2": consts["b2"],
        })
    res = run_bass_kernel_spmd(nc, in_maps, core_ids=list(range(NCORES)),
                               trace=_trace)
    J, Pp, cores_of = meta["J"], meta["Pp"], meta["cores_of"]
    out = np.zeros((N, D), np.float32)
    rows = J * P + Pp
    for c in range(NCORES):
        vs = np.where(cores_of == np.int64(c))[0]
        out[vs] = res.results[c]["out"][rows[vs]]
    kernel.last_exec_time_ns = res.exec_time_ns
    return out
